# revision 1
# baseline (speedup 1.0000x reference)
"""Fused DDPM dynamic-conv kernel for TRN2 (8 NeuronCores).

Math (reference):
  kernels = einsum('nchw,oc->nohw', y, gen_w) + gen_b        # o = d*576 + c*9 + t
  r_d     = sum_t kernels[d,c,t] * shift(x, tap t, dil d)    # d in {1,3,5}
  out     = conv3x3([x, r1, r3, r5], fuse_w) + fuse_b

Sharding: 8 cores = 4 batches x 2 H-halves (48 output rows each).
Per core, the 50 kern rows (48 + 1 halo each side) are split into two
26-row blocks (2-row overlap) packed on SBUF partitions: p = 64*blk + c.
All tap shifts are free-dim AP offsets into a zero-padded x tile.
Matmuls are full 128-wide with BLOCK-DIAGONAL weights so one matmul
serves both blocks (tiled <128 matmuls only support one sync wait in
walrus codegen).  The generator is reorganised per (d,t) so kernels land
in [c, pix] layout; gen_b is fused into the product via
scalar_tensor_tensor (PSUM source):
  prod = (kern_psum + gen_b[c]) * x_shifted     (one DVE op)
Tap accumulation is split DVE/GPSIMD.  The fuse conv is 9 taps x 4
groups of block-diag K=128 matmuls accumulating in PSUM; fuse_b is added
by the ACT Identity copy out of PSUM.  Matmuls use float32r (1 cyc/row).
"""

import numpy as np

K = 3
NB, C, H, W = 4, 64, 96, 96
NCORES = 8
HH = 48            # output rows per core
BLK = 26           # kern rows per block (24 out + 1 halo + 1 overlap)
XR = BLK + 10      # x rows per block (halo 5 each side for dil 5)
WP = W + 10        # padded width for x
RW = W + 2         # padded width for racc
DILS = (1, 3, 5)
CHUNKS = ((0, 10), (10, 10), (20, 6))  # kern-row chunks (start, nrows)
FCHUNKS = ((1, 5), (6, 5), (11, 5), (16, 5), (21, 4))  # out-row chunks
PUMP_N = 1
FUSE_PARTS = 3
PPOOL_BUFS = 4
GP_TAPS = ()           # tap products folded into racc on GPSIMD (hurts: delays fuse)
PE_TAPS = tuple(t for t in range(9) if t not in GP_TAPS)

_built = None
DEBUG_RACC = False


def _build():
    import concourse.mybir as mybir
    from concourse import bacc
    from concourse.tile import TileContext

    f32 = mybir.dt.float32
    f32r = mybir.dt.float32r
    add = mybir.AluOpType.add
    mult = mybir.AluOpType.mult
    ident = mybir.ActivationFunctionType.Identity

    nc = bacc.Bacc()
    xh = nc.dram_tensor("xh", [C, 60, WP], f32r, kind="ExternalInput")
    yh = nc.dram_tensor("yh", [C, 50, W], f32r, kind="ExternalInput")
    wg = nc.dram_tensor("wg", [128, 27 * 128], f32r, kind="ExternalInput")
    gb = nc.dram_tensor("gb", [128, 27], f32, kind="ExternalInput")
    fw = nc.dram_tensor("fw", [128, 9 * 4 * 128], f32r, kind="ExternalInput")
    fb = nc.dram_tensor("fb", [128, 1], f32, kind="ExternalInput")
    rm = nc.dram_tensor("rm", [128, 2], f32, kind="ExternalInput")
    ey = nc.dram_tensor("ey", [128, 128], f32r, kind="ExternalInput")
    out = nc.dram_tensor("out", [C, HH, W], f32, kind="ExternalOutput")
    if DEBUG_RACC:
        dbg = nc.dram_tensor("dbg", [128, 3, BLK, RW], f32, kind="ExternalOutput")

    with TileContext(nc) as tc:
        with (
            tc.tile_pool(name="const", bufs=1) as cpool,
            tc.tile_pool(name="prod", bufs=PPOOL_BUFS) as ppool,
            tc.tile_pool(name="kpsum", bufs=2, space="PSUM") as kpool,
            tc.tile_pool(name="rpsum", bufs=1, space="PSUM") as rpool,
            tc.tile_pool(name="fpsum", bufs=2, space="PSUM") as fpool,
        ):
            xpad = cpool.tile([128, XR, WP], f32r)
            ysb = cpool.tile([128, BLK * W], f32r)
            wgsb = cpool.tile([128, 27 * 128], f32r)
            gbsb = cpool.tile([128, 27], f32)
            fwsb = cpool.tile([128, 9 * 4 * 128], f32r)
            fbsb = cpool.tile([128, 1], f32)
            rmsb = cpool.tile([128, 2], f32)
            eysb = cpool.tile([128, 128], f32r)
            racc = cpool.tile([128, 3, BLK, RW], f32r)
            osb = cpool.tile([128, 24, W], f32)

            # --- loads + padding (x arrives pre-padded in W) ---
            # zero the 1-col borders of racc (cols 0 and 97)
            nc.vector.memset(racc[:, :, :, 0:RW:RW - 1].bitcast(f32), 0.0)
            # load order matters: the first gen matmuls need wg cols 0:384
            # and ysb; fuse weights are needed much later.
            ys3 = ysb[:].rearrange("p (r w) -> p r w", r=BLK)
            nc.sync.dma_start(out=ys3[0:64, 0:10, :], in_=yh[:, 0:10, :])
            nc.sync.dma_start(out=ys3[64:128, 0:10, :], in_=yh[:, 24:34, :])
            nc.sync.dma_start(out=wgsb[:, 0:384], in_=wg[:, 0:384])
            nc.sync.dma_start(out=gbsb[:, :], in_=gb[:, :])
            nc.sync.dma_start(out=eysb[:, :], in_=ey[:, :])
            nc.sync.dma_start(out=xpad[0:64, 0:21, :], in_=xh[:, 0:21, :])
            nc.sync.dma_start(out=xpad[64:128, 0:21, :], in_=xh[:, 24:45, :])
            nc.sync.dma_start(out=ys3[0:64, 10:BLK, :], in_=yh[:, 10:BLK, :])
            nc.sync.dma_start(out=ys3[64:128, 10:BLK, :], in_=yh[:, 34:50, :])
            nc.sync.dma_start(out=xpad[0:64, 21:XR, :], in_=xh[:, 21:XR, :])
            nc.sync.dma_start(out=xpad[64:128, 21:XR, :], in_=xh[:, 45:24 + XR, :])
            nc.sync.dma_start(out=wgsb[:, 384:1728], in_=wg[:, 384:1728])
            nc.sync.dma_start(out=wgsb[:, 1728:], in_=wg[:, 1728:])
            nc.sync.dma_start(out=rmsb[:, :], in_=rm[:, :])
            nc.sync.dma_start(out=fwsb[:, 0:2304], in_=fw[:, 0:2304])
            nc.sync.dma_start(out=fwsb[:, 2304:], in_=fw[:, 2304:])
            nc.sync.dma_start(out=fbsb[:, :], in_=fb[:, :])

            fuse_emitted = 0
            fuse_state = {}

            def fuse_unit(o0, nr, part):
                # 12 of the 36 accumulating matmuls for one out-row chunk;
                # part 0 allocates the psum tile, part 2 copies out via ACT
                if part == 0:
                    fp = fpool.tile([128, 5, W], f32, tag="fp")
                    fuse_state[o0] = fp
                fp = fuse_state[o0]
                ps = fp[:, 0:nr, :]
                items = [(di, dj, g) for di in (-1, 0, 1) for dj in (-1, 0, 1)
                         for g in range(4)]
                usz = 36 // FUSE_PARTS
                for cnt in range(part * usz, part * usz + usz):
                    di, dj, g = items[cnt]
                    ij = (di + 1) * 3 + (dj + 1)
                    if g == 0:
                        rhs = xpad[:, o0 + di + 5:o0 + di + 5 + nr,
                                   5 + dj:5 + dj + W]
                    else:
                        rhs = racc[:, g - 1, o0 + di:o0 + di + nr,
                                   1 + dj:1 + dj + W]
                    nc.tensor.matmul(
                        ps, fwsb[:, (ij * 4 + g) * 128:(ij * 4 + g + 1) * 128],
                        rhs, start=(cnt == 0), stop=(cnt == 35),
                    )
                if part == FUSE_PARTS - 1:
                    nc.scalar.activation(
                        osb[:, o0 - 1:o0 - 1 + nr, :], ps, ident,
                        bias=fbsb[:, 0:1])
                    del fuse_state[o0]
                    nc.sync.dma_start(out=out[:, o0 - 1:o0 - 1 + nr, :],
                                      in_=osb[0:64, o0 - 1:o0 - 1 + nr, :])
                    nc.sync.dma_start(out=out[:, 23 + o0:23 + o0 + nr, :],
                                      in_=osb[64:128, o0 - 1:o0 - 1 + nr, :])

            fuse_q = []

            def pump_fuse(maxn):
                n = 0
                while fuse_q and n < maxn:
                    o0, nr, part = fuse_q.pop(0)
                    fuse_unit(o0, nr, part)
                    n += 1

            fuse_emitted = 0
            for ci, (r0, nrc) in enumerate(CHUNKS):
                nb = nrc // 2   # rows per PSUM bank (uniform 2-bank tiles)
                for dd, d in enumerate(DILS):
                    rp = rpool.tile([128, 2, 512], f32, tag="rp")
                    gp_prs = []
                    for t in range(9):
                        di, dj = t // 3 - 1, t % 3 - 1
                        dt = dd * 9 + t
                        kp = kpool.tile([128, 2, 512], f32, tag="kp")
                        # each matmul writes within a single PSUM bank
                        for k in (0, 1):
                            nc.tensor.matmul(
                                kp[:, k, 0:nb * W],
                                wgsb[:, dt * 128:(dt + 1) * 128],
                                ysb[:, (r0 + k * nb) * W:(r0 + (k + 1) * nb) * W],
                                start=True, stop=True,
                            )
                        kv = kp[:, :, 0:nb * W].rearrange(
                            "p b (r w) -> p b r w", w=W)
                        x0 = r0 + di * d + 5
                        xs = xpad[:, x0:x0 + nrc, 5 + dj * d:5 + dj * d + W
                                  ].rearrange("p (b r) w -> p b r w", r=nb)
                        pr = ppool.tile([128, 10, W], f32r, tag="pr")
                        prv = pr[:, 0:nrc, :].rearrange(
                            "p (b r) w -> p b r w", r=nb)
                        nc.vector.scalar_tensor_tensor(
                            prv, kv, gbsb[:, dt:dt + 1], xs, add, mult)
                        # accumulate the 9 taps on the PE: rp += I @ pr
                        for k in (0, 1):
                            nc.tensor.matmul(
                                rp[:, k, 0:nb * W],
                                eysb[:, :],
                                pr[:, k * nb:(k + 1) * nb, :],
                                start=(t == 0), stop=(t == 8),
                            )
                    # r_d chunk -> SBUF racc (ACT), rounding to f32r
                    rv = racc[:, dd, r0:r0 + nrc, 1:1 + W].rearrange(
                        "p (b r) w -> p b r w", r=nb)
                    nc.scalar.copy(
                        rv, rp[:, :, 0:nb * W].rearrange(
                            "p b (r w) -> p b r w", w=W))
                    # zero out-of-image halo rows (reference zero-pads cat):
                    # row 0 of block A when h0==0, row 25 of block B when
                    # h0==48 -- per-core masks keep the program SPMD-uniform
                    if r0 == 0:
                        nc.vector.tensor_scalar_mul(
                            racc[:, dd, 0, 1:1 + W], racc[:, dd, 0, 1:1 + W],
                            rmsb[:, 0:1])
                    elif r0 + nrc == BLK:
                        nc.vector.tensor_scalar_mul(
                            racc[:, dd, BLK - 1, 1:1 + W],
                            racc[:, dd, BLK - 1, 1:1 + W], rmsb[:, 1:2])
                    pump_fuse(PUMP_N)
                # queue fuse chunks whose racc rows are fully computed
                while fuse_emitted < len(FCHUNKS):
                    o0, nr = FCHUNKS[fuse_emitted]
                    if o0 + nr + 1 > r0 + nrc:
                        break
                    fuse_q.extend([(o0, nr, p) for p in range(FUSE_PARTS)])
                    fuse_emitted += 1
            while fuse_emitted < len(FCHUNKS):
                    o0, nr = FCHUNKS[fuse_emitted]
                    if o0 + nr + 1 > r0 + nr13:
                        break
                    emit_fuse(o0, nr)
                    fuse_emitted += 1
            while fuse_emitted < len(FCHUNKS):
                o0, nr = FCHUNKS[fuse_emitted]
                fuse_q.extend([(o0, nr, p) for p in range(FUSE_PARTS)])
                fuse_emitted += 1
            pump_fuse(len(fuse_q))

            if DEBUG_RACC:
                nc.sync.dma_start(out=dbg[:, :, :, :],
                                  in_=racc[:, :, :, :].bitcast(f32))
    nc.finalize()
    return nc


def _prep_inputs(x, y, gen_w, gen_b, fuse_w, fuse_b):
    # generator weights: W_dt[c', c] = gen_w[d*576 + c*9 + t, c'],
    # block-diagonal over the two H-blocks.
    w3 = gen_w.reshape(3, 64, 9, 64).transpose(3, 0, 2, 1).reshape(64, 27, 64)
    wgh = np.zeros((128, 27, 128), np.float32)
    wgh[0:64, :, 0:64] = w3
    wgh[64:128, :, 64:128] = w3
    wgh = np.ascontiguousarray(wgh.reshape(128, 27 * 128))
    gbh = gen_b.reshape(3, 64, 9).transpose(1, 0, 2).reshape(64, 27)
    gbh = np.ascontiguousarray(np.concatenate([gbh, gbh], 0))
    # fuse weights: [k, ij, g, o] block-diagonal
    f3 = fuse_w.transpose(1, 2, 3, 0).reshape(4, 64, 9, 64).transpose(1, 2, 0, 3)
    fwh = np.zeros((64, 9, 4, 128), np.float32)
    fwh[:, :, :, 0:64] = f3
    fwh2 = np.zeros((128, 9, 4, 128), np.float32)
    fwh2[0:64] = fwh
    fwh2[64:128, :, :, 64:128] = f3
    fwh2[64:128, :, :, 0:64] = 0.0
    fwh = np.ascontiguousarray(fwh2.reshape(128, 9 * 4 * 128))
    fbh = np.ascontiguousarray(np.concatenate([fuse_b, fuse_b])[:, None])
    xp = np.pad(x, ((0, 0), (0, 0), (6, 6), (5, 5)))
    yp = np.pad(y, ((0, 0), (0, 0), (1, 1), (0, 0)))
    maps = []
    for c in range(NCORES):
        n, half = c // 2, c % 2
        h0 = HH * half
        rmh = np.ones((128, 2), np.float32)
        if half == 0:
            rmh[0:64, 0] = 0.0     # block A row 0 = global row -1
        else:
            rmh[64:128, 1] = 0.0   # block B row 25 = global row 96
        maps.append({
            "xh": np.ascontiguousarray(xp[n, :, h0:h0 + 60, :]),
            "yh": np.ascontiguousarray(yp[n, :, h0:h0 + 50, :]),
            "wg": wgh, "gb": gbh, "fw": fwh, "fb": fbh, "rm": rmh,
            "ey": np.eye(128, dtype=np.float32),
        })
    return maps


def kernel(x, y, gen_w, gen_b, fuse_w, fuse_b):
    global _built
    from concourse.bass_utils import run_bass_kernel_spmd

    x = np.asarray(x, np.float32)
    y = np.asarray(y, np.float32)
    gen_w = np.asarray(gen_w, np.float32)
    gen_b = np.asarray(gen_b, np.float32)
    fuse_w = np.asarray(fuse_w, np.float32)
    fuse_b = np.asarray(fuse_b, np.float32)

    if _built is None:
        _built = _build()
    maps = _prep_inputs(x, y, gen_w, gen_b, fuse_w, fuse_b)
    res = run_bass_kernel_spmd(_built, maps, list(range(NCORES)))
    outf = np.empty((NB, C, H, W), np.float32)
    for c in range(NCORES):
        n, half = c // 2, c % 2
        outf[n, :, HH * half:HH * half + HH, :] = res.results[c]["out"]
    return outf

